# revision 4
# baseline (speedup 1.0000x reference)
"""int4 group-quantized linear: y = x @ dequant(w_packed, w_scale, w_zero).T

Full shapes: x [4096, 4096] f32, W [11008, 4096] int4 (group=128),
y [4096, 11008] f32.

Strategy: column-parallel over 8 NeuronCores. Each core handles 1376
out-features (zero-padded to 1408 = 11*128):
  - dequant packed nibbles on DVE in natural [o, i] layout
    (xor/shift/and extraction + fused per-group (nib*s - (8+z)*s) affine
    with per-partition AP scalars, int32 -> bf16)
  - PE-transpose dequantized W tiles into W.T [i, o] bf16, SBUF-resident
  - per 128-token tile: SWDGE DMA-cast x f32->bf16, PE-transpose to
    x.T tiles; matmul with lhsT=x.T[g] stationary, rhs=W.T[g] moving,
    accumulating 32 k-tiles in PSUM -> y tile in natural [t, o] layout
"""

import numpy as np

import concourse.bacc as bacc
import concourse.bass as bass
import concourse.mybir as mybir
import concourse.tile as tile
from concourse.bass_utils import run_bass_kernel_spmd
from concourse.masks import make_identity

OUT, IN, TOK, GROUP = 11008, 4096, 4096, 128
NG = IN // GROUP          # 32 groups (= k-tiles)
NCORES = 8
OSH = OUT // NCORES       # 1376 real out-features per core
OTILES = (OSH + 127) // 128   # 11
OPAD = OTILES * 128       # 1408
ROW_BYTES = IN // 2       # 2048 packed bytes per out-feature row
TTILES = TOK // 128       # 32 token tiles
# matmul N-chunks over the padded out dim
OCHUNKS = [(0, 512), (512, 512), (1024, OPAD - 1024)]

F32 = mybir.dt.float32
BF16 = mybir.dt.bfloat16
I32 = mybir.dt.int32
ALU = mybir.AluOpType


def build(nc: bass.Bass):
    x_d = nc.dram_tensor("x", (TOK, IN), F32, kind="ExternalInput")
    wp_d = nc.dram_tensor("wp", (OPAD, ROW_BYTES), I32, kind="ExternalInput")
    ws_d = nc.dram_tensor("ws", (OPAD, NG), F32, kind="ExternalInput")
    wz_d = nc.dram_tensor("wz", (OPAD, NG), I32, kind="ExternalInput")
    y_d = nc.dram_tensor("y", (TOK, OPAD), F32, kind="ExternalOutput")

    with tile.TileContext(nc) as tc:
        with tc.tile_pool(name="singles", bufs=1) as singles, \
             tc.tile_pool(name="wtpool", bufs=1) as wtpool, \
             tc.tile_pool(name="prep", bufs=2) as prep, \
             tc.tile_pool(name="prep1", bufs=1) as prep1, \
             tc.tile_pool(name="xpool", bufs=2) as xpool, \
             tc.tile_pool(name="ypool", bufs=6) as ypool, \
             tc.tile_pool(name="psA", bufs=4, space="PSUM") as psA, \
             tc.tile_pool(name="psB", bufs=2, space="PSUM") as psB, \
             tc.tile_pool(name="psW", bufs=2, space="PSUM") as psW:

            ident = singles.tile([128, 128], BF16)
            make_identity(nc, ident)

            # W.T resident: [128 i-part, g-major: g*OPAD + o] bf16
            wt = wtpool.tile([128, NG * OPAD], BF16)
            wt_g = wt.rearrange("p (g o) -> p g o", g=NG)

            # ---------------- W prep: dequant + transpose ----------------
            for ot in range(OTILES):
                s_sb = prep.tile([128, NG], F32)
                z_sb = prep.tile([128, NG], I32)
                nc.sync.dma_start(out=s_sb, in_=ws_d.ap()[ot * 128:(ot + 1) * 128, :])
                nc.sync.dma_start(out=z_sb, in_=wz_d.ap()[ot * 128:(ot + 1) * 128, :])
                # zs8 = (z + 8) * s
                z8 = prep.tile([128, NG], F32)
                nc.vector.tensor_scalar(out=z8, in0=z_sb, scalar1=8.0, scalar2=None,
                                        op0=ALU.add)
                zs8 = prep.tile([128, NG], F32)
                nc.vector.tensor_tensor(out=zs8, in0=z8, in1=s_sb, op=ALU.mult)

                wp_sb = prep.tile([128, ROW_BYTES], I32)
                nc.sync.dma_start(out=wp_sb,
                                  in_=wp_d.ap()[ot * 128:(ot + 1) * 128, :])

                # nibble extraction (interleaved writes, safe path)
                w4x = prep1.tile([128, IN], I32)
                nc.vector.tensor_scalar(
                    out=w4x[:, 0:IN:2], in0=wp_sb, scalar1=0x8, scalar2=15,
                    op0=ALU.bitwise_xor, op1=ALU.bitwise_and)
                nc.vector.tensor_scalar(
                    out=w4x[:, 1:IN:2], in0=wp_sb, scalar1=0x88, scalar2=4,
                    op0=ALU.bitwise_xor, op1=ALU.logical_shift_right)

                # per-group affine: W = w4 * s[:,g] - (8+z[:,g])*s[:,g]
                w_bf = prep.tile([128, IN], BF16)
                for g in range(NG):
                    nc.vector.tensor_scalar(
                        out=w_bf[:, g * 128:(g + 1) * 128],
                        in0=w4x[:, g * 128:(g + 1) * 128],
                        scalar1=s_sb[:, g:g + 1], scalar2=zs8[:, g:g + 1],
                        op0=ALU.mult, op1=ALU.subtract)

                # transpose [o, i] -> [i, o]; batch 4 groups per PSUM tile
                for gq in range(NG // 4):
                    tpw = psW.tile([128, 512], BF16)
                    for j in range(4):
                        g = gq * 4 + j
                        nc.tensor.transpose(
                            tpw[:, j * 128:(j + 1) * 128],
                            w_bf[:, g * 128:(g + 1) * 128], ident)
                    nc.scalar.copy(
                        out=wt_g[:, gq * 4:(gq + 1) * 4,
                                 ot * 128:(ot + 1) * 128],
                        in_=tpw.rearrange("p (j o) -> p j o", j=4))

            # ---------------- main loop over token tiles ----------------
            for tt in range(TTILES):
                x_bf = xpool.tile([128, IN], BF16)
                # SWDGE cast f32 -> bf16 during DMA
                nc.gpsimd.dma_start(
                    out=x_bf, in_=x_d.ap()[tt * 128:(tt + 1) * 128, :])

                xt = xpool.tile([128, NG * 128], BF16)
                for gq in range(NG // 4):
                    tpx = psB.tile([128, 512], BF16)
                    for j in range(4):
                        g = gq * 4 + j
                        nc.tensor.transpose(
                            tpx[:, j * 128:(j + 1) * 128],
                            x_bf[:, g * 128:(g + 1) * 128], ident)
                    nc.scalar.copy(out=xt[:, gq * 512:(gq + 1) * 512], in_=tpx)

                yps = []
                for oc, (o0, n) in enumerate(OCHUNKS):
                    yp = psA.tile([128, 512], F32, name="yp", tag="yp")
                    yps.append(yp)
                for g in range(NG):
                    for oc, (o0, n) in enumerate(OCHUNKS):
                        nc.tensor.matmul(
                            yps[oc][:, :n],
                            xt[:, g * 128:(g + 1) * 128],
                            wt[:, g * OPAD + o0: g * OPAD + o0 + n],
                            start=(g == 0), stop=(g == NG - 1))
                for oc, (o0, n) in enumerate(OCHUNKS):
                    y_sb = ypool.tile([128, 512], F32, name="ysb", tag="ysb")
                    nc.scalar.copy(out=y_sb[:, :n], in_=yps[oc][:, :n])
                    nc.sync.dma_start(
                        out=y_d.ap()[tt * 128:(tt + 1) * 128, o0:o0 + n],
                        in_=y_sb[:, :n])


_nc_cache = None


def _get_nc():
    global _nc_cache
    if _nc_cache is None:
        nc = bacc.Bacc("TRN2", target_bir_lowering=False, debug=False)
        build(nc)
        nc.compile()
        _nc_cache = nc
    return _nc_cache


def make_in_maps(x, w_packed, w_scale, w_zero):
    x = np.ascontiguousarray(np.asarray(x, dtype=np.float32))
    wp = np.asarray(w_packed, dtype=np.int32).reshape(OUT, ROW_BYTES)
    ws = np.asarray(w_scale, dtype=np.float32)
    wz = np.asarray(w_zero, dtype=np.int32)

    in_maps = []
    for c in range(NCORES):
        sl = slice(c * OSH, (c + 1) * OSH)
        wp_c = np.zeros((OPAD, ROW_BYTES), dtype=np.int32)
        wp_c[:OSH] = wp[sl]
        ws_c = np.zeros((OPAD, NG), dtype=np.float32)
        ws_c[:OSH] = ws[sl]
        wz_c = np.zeros((OPAD, NG), dtype=np.int32)
        wz_c[:OSH] = wz[sl]
        in_maps.append({"x": x, "wp": wp_c, "ws": ws_c, "wz": wz_c})
    return in_maps


def kernel(x, w_packed, w_scale, w_zero):
    nc = _get_nc()
    in_maps = make_in_maps(x, w_packed, w_scale, w_zero)
    res = run_bass_kernel_spmd(nc, in_maps, core_ids=list(range(NCORES)))
    y = np.concatenate([res.results[c]["y"][:, :OSH] for c in range(NCORES)],
                       axis=1)
    return y.astype(np.float32)


# revision 7
# speedup vs baseline: 1.4846x; 1.4846x over previous
"""int4 group-quantized linear: y = x @ dequant(w_packed, w_scale, w_zero).T

Full shapes: x [4096, 4096] f32, W [11008, 4096] int4 (group=128),
y [4096, 11008] f32.

Strategy: column-parallel over 8 NeuronCores. Each core handles 1376
out-features (zero-padded to 1408 = 11*128):
  - dequant packed nibbles on DVE in natural [o, i] layout
    (xor/shift/and extraction + fused per-group (nib*s - (8+z)*s) affine
    with per-partition AP scalars, int32 -> bf16)
  - PE-transpose dequantized W tiles into W.T [i, o] bf16, SBUF-resident
  - per 128-token tile: SWDGE DMA-cast x f32->bf16, PE-transpose to
    x.T tiles; matmul with lhsT=x.T[g] stationary, rhs=W.T[g] moving,
    accumulating 32 k-tiles in PSUM -> y tile in natural [t, o] layout
"""

import numpy as np

import concourse.bacc as bacc
import concourse.bass as bass
import concourse.mybir as mybir
import concourse.tile as tile
from concourse.bass_utils import run_bass_kernel_spmd
from concourse.masks import make_identity

OUT, IN, TOK, GROUP = 11008, 4096, 4096, 128
NG = IN // GROUP          # 32 groups (= k-tiles)
NCORES = 8
OSH = OUT // NCORES       # 1376 real out-features per core
OTILES = (OSH + 127) // 128   # 11
OPAD = OTILES * 128       # 1408
ROW_BYTES = IN // 2       # 2048 packed bytes per out-feature row
TTILES = TOK // 128       # 32 token tiles
# matmul N-chunks over the padded out dim
OCHUNKS = [(0, 512), (512, 512), (1024, OPAD - 1024)]

F32 = mybir.dt.float32
BF16 = mybir.dt.bfloat16
I32 = mybir.dt.int32
ALU = mybir.AluOpType


def build(nc: bass.Bass, variant: str = "base"):
    x_d = nc.dram_tensor("x", (TOK, IN), F32, kind="ExternalInput")
    wp_d = nc.dram_tensor("wp", (OPAD, ROW_BYTES), I32, kind="ExternalInput")
    ws_d = nc.dram_tensor("ws", (OPAD, NG), F32, kind="ExternalInput")
    wz_d = nc.dram_tensor("wz", (OPAD, NG), I32, kind="ExternalInput")
    y_d = nc.dram_tensor("y", (TOK, OPAD), F32, kind="ExternalOutput")

    with tile.TileContext(nc) as tc:
        with tc.tile_pool(name="singles", bufs=1) as singles, \
             tc.tile_pool(name="wtpool", bufs=1) as wtpool, \
             tc.tile_pool(name="prep", bufs=2) as prep, \
             tc.tile_pool(name="prep1", bufs=1) as prep1, \
             tc.tile_pool(name="xpool", bufs=2) as xpool, \
             tc.tile_pool(name="ypool", bufs=6) as ypool, \
             tc.tile_pool(name="psA", bufs=4, space="PSUM") as psA, \
             tc.tile_pool(name="psB", bufs=2, space="PSUM") as psB, \
             tc.tile_pool(name="psW", bufs=2, space="PSUM") as psW:

            ident = singles.tile([128, 128], BF16)
            make_identity(nc, ident)

            # W.T resident: [128 i-part, g-major: g*OPAD + o] bf16
            wt = wtpool.tile([128, NG * OPAD], BF16)
            wt_g = wt.rearrange("p (g o) -> p g o", g=NG)

            xt_fixed = None
            if variant == "nox":
                xt_fixed = singles.tile([128, NG * 128], BF16)
                nc.vector.memset(xt_fixed, 0.5)

            # ---------------- W prep: dequant + transpose ----------------
            for ot in range(OTILES if variant != "noprep" else 0):
                s_sb = prep.tile([128, NG], F32)
                z_sb = prep.tile([128, NG], I32)
                nc.sync.dma_start(out=s_sb, in_=ws_d.ap()[ot * 128:(ot + 1) * 128, :])
                nc.sync.dma_start(out=z_sb, in_=wz_d.ap()[ot * 128:(ot + 1) * 128, :])
                # zs8 = (z + 8) * s
                z8 = prep.tile([128, NG], F32)
                nc.vector.tensor_scalar(out=z8, in0=z_sb, scalar1=8.0, scalar2=None,
                                        op0=ALU.add)
                zs8 = prep.tile([128, NG], F32)
                nc.vector.tensor_tensor(out=zs8, in0=z8, in1=s_sb, op=ALU.mult)

                wp_sb = prep.tile([128, ROW_BYTES], I32)
                nc.sync.dma_start(out=wp_sb,
                                  in_=wp_d.ap()[ot * 128:(ot + 1) * 128, :])

                # nibble extraction (interleaved writes, safe path)
                w4x = prep1.tile([128, IN], I32)
                nc.vector.tensor_scalar(
                    out=w4x[:, 0:IN:2], in0=wp_sb, scalar1=0x8, scalar2=15,
                    op0=ALU.bitwise_xor, op1=ALU.bitwise_and)
                nc.vector.tensor_scalar(
                    out=w4x[:, 1:IN:2], in0=wp_sb, scalar1=0x88, scalar2=4,
                    op0=ALU.bitwise_xor, op1=ALU.logical_shift_right)

                # per-group affine: W = w4 * s[:,g] - (8+z[:,g])*s[:,g]
                w_bf = prep.tile([128, IN], BF16)
                for g in range(NG):
                    nc.vector.tensor_scalar(
                        out=w_bf[:, g * 128:(g + 1) * 128],
                        in0=w4x[:, g * 128:(g + 1) * 128],
                        scalar1=s_sb[:, g:g + 1], scalar2=zs8[:, g:g + 1],
                        op0=ALU.mult, op1=ALU.subtract)

                # transpose [o, i] -> [i, o]; batch 4 groups per PSUM tile
                for gq in range(NG // 4):
                    tpw = psW.tile([128, 512], BF16)
                    for j in range(4):
                        g = gq * 4 + j
                        nc.tensor.transpose(
                            tpw[:, j * 128:(j + 1) * 128],
                            w_bf[:, g * 128:(g + 1) * 128], ident)
                    nc.scalar.copy(
                        out=wt_g[:, gq * 4:(gq + 1) * 4,
                                 ot * 128:(ot + 1) * 128],
                        in_=tpw.rearrange("p (j o) -> p j o", j=4))

            if variant == "noprep":
                nc.gpsimd.memset(wt, 0.001)

            # ---------------- main loop over token tiles ----------------
            for tt in range(TTILES):
                if variant == "nox":
                    xt = xt_fixed
                else:
                    if variant == "syncx":
                        x_f32 = xpool.tile([128, IN], F32)
                        nc.sync.dma_start(
                            out=x_f32, in_=x_d.ap()[tt * 128:(tt + 1) * 128, :])
                        x_bf = xpool.tile([128, IN], BF16)
                        nc.vector.tensor_copy(out=x_bf, in_=x_f32)
                    else:
                        x_bf = xpool.tile([128, IN], BF16)
                        # SWDGE cast f32 -> bf16 during DMA
                        nc.gpsimd.dma_start(
                            out=x_bf, in_=x_d.ap()[tt * 128:(tt + 1) * 128, :])

                    xt = xpool.tile([128, NG * 128], BF16)
                    for gq in range(NG // 4):
                        tpx = psB.tile([128, 512], BF16)
                        for j in range(4):
                            g = gq * 4 + j
                            nc.tensor.transpose(
                                tpx[:, j * 128:(j + 1) * 128],
                                x_bf[:, g * 128:(g + 1) * 128], ident)
                        nc.scalar.copy(out=xt[:, gq * 512:(gq + 1) * 512],
                                       in_=tpx)
                if variant == "nomm":
                    continue

                yps = []
                for oc, (o0, n) in enumerate(OCHUNKS):
                    yp = psA.tile([128, 512], F32, name="yp", tag="yp")
                    yps.append(yp)
                for g in range(NG):
                    for oc, (o0, n) in enumerate(OCHUNKS):
                        nc.tensor.matmul(
                            yps[oc][:, :n],
                            xt[:, g * 128:(g + 1) * 128],
                            wt[:, g * OPAD + o0: g * OPAD + o0 + n],
                            start=(g == 0), stop=(g == NG - 1))
                for oc, (o0, n) in enumerate(OCHUNKS):
                    y_sb = ypool.tile([128, 512], F32, name="ysb", tag="ysb")
                    nc.scalar.copy(out=y_sb[:, :n], in_=yps[oc][:, :n])
                    nc.sync.dma_start(
                        out=y_d.ap()[tt * 128:(tt + 1) * 128, o0:o0 + n],
                        in_=y_sb[:, :n])


_nc_cache = None


def _get_nc():
    global _nc_cache
    if _nc_cache is None:
        nc = bacc.Bacc("TRN2", target_bir_lowering=False, debug=False)
        build(nc)
        nc.compile()
        _nc_cache = nc
    return _nc_cache


def make_in_maps(x, w_packed, w_scale, w_zero):
    x = np.ascontiguousarray(np.asarray(x, dtype=np.float32))
    wp = np.asarray(w_packed, dtype=np.int32).reshape(OUT, ROW_BYTES)
    ws = np.asarray(w_scale, dtype=np.float32)
    wz = np.asarray(w_zero, dtype=np.int32)

    in_maps = []
    for c in range(NCORES):
        sl = slice(c * OSH, (c + 1) * OSH)
        wp_c = np.zeros((OPAD, ROW_BYTES), dtype=np.int32)
        wp_c[:OSH] = wp[sl]
        ws_c = np.zeros((OPAD, NG), dtype=np.float32)
        ws_c[:OSH] = ws[sl]
        wz_c = np.zeros((OPAD, NG), dtype=np.int32)
        wz_c[:OSH] = wz[sl]
        in_maps.append({"x": x, "wp": wp_c, "ws": ws_c, "wz": wz_c})
    return in_maps


def kernel(x, w_packed, w_scale, w_zero):
    nc = _get_nc()
    in_maps = make_in_maps(x, w_packed, w_scale, w_zero)
    res = run_bass_kernel_spmd(nc, in_maps, core_ids=list(range(NCORES)))
    y = np.concatenate([res.results[c]["y"][:, :OSH] for c in range(NCORES)],
                       axis=1)
    return y.astype(np.float32)
